# revision 1
# baseline (speedup 1.0000x reference)
"""DialogueRNN forward on 8 Trainium2 NeuronCores (Bass/Tile, SPMD).

Strategy
--------
Data-parallel over batch: B=128 -> 16 per core; all weights replicated.
One SPMD program; every per-core difference (batch slice, speaker gather /
scatter indices) flows through input data.

Per core, three phases:
  1) Fusion + input-side precompute, batched over all T:
       utterT = WfT_ext.T @ xT            (bf folded via ones-row in x)
       Ug     = utter @ [Wgi_u | Wpi_u].T (+ summed GRU biases via ones-row)
     Ug is streamed back per scan step from DRAM.
  2) Sequential scan over T=256 steps. Recurrent matmuls use an
     activations-stationary / weights-moving float32r layout:
       out[16, 512] = lhsT[128, 16].T @ W[128, 512]   (1 cycle/row)
     Personal states live feature-major in an SBUF store [128, 9*4*16];
     speaker gather and scatter go through gpsimd.ap_gather with runtime
     index tiles (spk = argmax(party_mask) computed host-side). Only the
     speaker's personal state updates (the reference discards the other
     parties' GRU outputs). The history attention keeps the reference's
     online-softmax state (m, l, acc); ctx enters the personal GRU by
     scaling the acc lhsT columns with 1/l, which commutes through the
     matmul because it is a per-batch scalar.
  3) MatchingAttention head per batch lane (q x t attention over time),
     then Linear+ReLU+Linear+log_softmax.
"""

import sys

sys.path.insert(0, "/opt/trn_rl_repo")

import numpy as np
from contextlib import ExitStack

import concourse.tile as tile
from concourse import bacc
from concourse import mybir
from concourse.bass_utils import run_bass_kernel_spmd
from concourse.masks import make_identity

F32 = mybir.dt.float32
F32R = mybir.dt.float32r
I16 = mybir.dt.int16
AF = mybir.ActivationFunctionType
MUL = mybir.AluOpType.mult

T, B, P = 256, 128, 9
NCORES = 8
BC = B // NCORES          # 16 batch lanes per core
D = 512                   # Du = Dg = Dp = De = Dh
G = 3 * D                 # 1536 gate width
KT = D // 128             # 4 k-tiles per 512-wide contraction
DCAT = 600 + 300 + 300    # 1200
KF = 1280                 # padded fused-input contraction (1200 + ones + pad)
ROWS = T * BC             # 4096 rows per core
C = 7
C8 = 8                    # class dim padded to 8 (f32r moving N must be 4-aligned)
NEG = -1e9
NSTORE = P * KT * BC      # 576

# debug knobs (used by dev tests only; grading uses defaults)
DEBUG_OUTS = ()      # subset of {"ug", "emo"} exposed as outputs (dev only)
RUN_SCAN = True
RUN_HEAD = True
SCAN_PARTS = frozenset(("gather", "attn", "p", "e"))


def _mm_gru(nc, ps_rz, ps_ni, ps_nh, lhsT_i, w_i, lhsT_h, w_h):
    """The 24 matmuls of one GRU step.

    ps_rz [BC, 2, 512]: r,z pre-activations; i-side and h-side accumulate
    into the same banks. ps_ni / ps_nh [BC, 512]: the n-gate parts stay
    separate (n = tanh(i_n + r * h_n)).
    """
    for n in range(2):
        for k in range(KT):
            nc.tensor.matmul(
                ps_rz[:, n, :], lhsT_i[:, k, :], w_i[:, k, n * D:(n + 1) * D],
                start=(k == 0), stop=False,
            )
        for k in range(KT):
            nc.tensor.matmul(
                ps_rz[:, n, :], lhsT_h[:, k, :], w_h[:, k, n * D:(n + 1) * D],
                start=False, stop=(k == KT - 1),
            )
    for k in range(KT):
        nc.tensor.matmul(
            ps_ni, lhsT_i[:, k, :], w_i[:, k, 2 * D:],
            start=(k == 0), stop=(k == KT - 1),
        )
    for k in range(KT):
        nc.tensor.matmul(
            ps_nh, lhsT_h[:, k, :], w_h[:, k, 2 * D:],
            start=(k == 0), stop=(k == KT - 1),
        )


def _transpose_to(nc, psum_pool, ident, src, dst):
    """src [BC, 512] batch-major -> dst [128, KT, BC] feature-major."""
    trp = psum_pool.tile([128, KT, BC], F32, tag="ni", bufs=2)
    for k in range(KT):
        nc.tensor.transpose(trp[:, k, :], src[:, k * 128:(k + 1) * 128],
                            ident[:BC, :BC])
    nc.vector.tensor_copy(dst, trp)


def _bcast16(ap):
    # [128, BC] -> [128, KT, BC] with a stride-0 middle dim
    return ap.rearrange("p (o b) -> p o b", o=1).broadcast_to((128, KT, BC))


def build_program(add_ebias):
    nc = bacc.Bacc("TRN2", target_bir_lowering=False, debug=False,
                   num_devices=NCORES)

    def din(name, shape, dt=F32):
        return nc.dram_tensor(name, shape, dt, kind="ExternalInput").ap()

    xT_d = din("xT", [KF, ROWS])
    wf_d = din("wf", [KF, D])
    wu_d = din("wu", [D, 2 * G])
    sb_d = din("sb", [1, 2 * G])
    wdrams = {nm: din(nm, [D, G])
              for nm in ("wsp", "wgh", "wpic", "wph", "wei", "weh")}
    wa_d = din("wa", [128, KT])
    gidx_d = din("gidx", [128, T * KT], I16)
    rbidx_d = din("rbidx", [T, 128, P * KT], I16)
    wm_d = din("wm", [D, D])
    bm_d = din("bm", [1, D])
    wl_d = din("wl", [D, D])
    bl_d = din("bl", [1, D])
    ws_d = din("ws", [D, C8])
    bs_d = din("bs", [1, C8])
    if add_ebias:
        eb_d = din("ebias", [1, G])

    ug_d = nc.dram_tensor(
        "ug_store", [ROWS, 2 * G], F32,
        kind="ExternalOutput" if "ug" in DEBUG_OUTS else "Internal").ap()
    emo_d = nc.dram_tensor(
        "emo_store", [ROWS, D], F32,
        kind="ExternalOutput" if "emo" in DEBUG_OUTS else "Internal").ap()
    out_d = nc.dram_tensor("out", [ROWS, C], F32, kind="ExternalOutput").ap()

    def r128(ap, inner):
        # [K*128, inner] DRAM view -> [128, K, inner] partition-major
        return ap.rearrange("(k p) n -> p k n", p=128)

    with ExitStack() as ctx:
        tc = ctx.enter_context(tile.TileContext(nc))
        ctx.enter_context(nc.allow_low_precision(
            reason="deliberate float32r rounding of matmul operands"))

        const = ctx.enter_context(tc.tile_pool(name="const", bufs=1))
        state = ctx.enter_context(tc.tile_pool(name="state", bufs=1))

        ident = const.tile([128, 128], F32)
        make_identity(nc, ident)
        identr = const.tile([128, 128], F32R)
        nc.vector.tensor_copy(identr, ident)
        ones_f = const.tile([1, max(T, 128)], F32)
        nc.vector.memset(ones_f, 1.0)
        ones_col = const.tile([1, 128], F32R)
        nc.vector.tensor_copy(ones_col, ones_f[:, :128])
        onesT = const.tile([1, T], F32R)
        nc.vector.tensor_copy(onesT, ones_f[:, :T])
        wa_sb = const.tile([128, KT], F32R)
        nc.sync.dma_start(out=wa_sb, in_=wa_d[:].bitcast(F32R))
        gidx_sb = const.tile([128, T * KT], I16)
        nc.sync.dma_start(out=gidx_sb, in_=gidx_d[:])
        if add_ebias:
            eb_sb = const.tile([BC, G], F32)
            nc.sync.dma_start(out=eb_sb, in_=eb_d[:].to_broadcast((BC, G)))

        # persistent scan state
        gT = state.tile([128, KT, BC], F32R)      # global state, feature-major
        g_b = state.tile([BC, D], F32)            # global state, batch-major
        eT = state.tile([128, KT, BC], F32R)
        emo_b = state.tile([BC, D], F32)
        accT = state.tile([128, KT, BC], F32R)
        m_sb = state.tile([1, BC], F32)
        l_sb = state.tile([1, BC], F32)
        pstA = state.tile([128, NSTORE + KT * BC], F32)  # store + staging
        pstB = state.tile([128, NSTORE + KT * BC], F32)
        zro = const.tile([128, NSTORE + KT * BC], F32)
        nc.vector.memset(zro, 0.0)
        for st in (gT, eT, accT):
            nc.vector.tensor_copy(st.rearrange("p k b -> p (k b)"),
                                  zro[:, :KT * BC])
        nc.vector.memset(pstA, 0.0)
        nc.vector.memset(pstB, 0.0)
        for st in (g_b, emo_b, l_sb):
            nc.vector.memset(st, 0.0)
        nc.vector.memset(m_sb, NEG)

        # ---------------- phase 1: fusion + precompute ----------------
        with ExitStack() as p1:
            p1sb = p1.enter_context(tc.tile_pool(name="p1sb", bufs=1))
            p1w = p1.enter_context(tc.tile_pool(name="p1w", bufs=2))
            p1ps = p1.enter_context(tc.tile_pool(name="p1ps", bufs=1,
                                                 space="PSUM"))

            wf_sb = p1sb.tile([128, KF // 128, D], F32R)
            nc.sync.dma_start(out=wf_sb, in_=r128(wf_d, D).bitcast(F32R))
            wu_sb = p1sb.tile([128, KT, 2 * G], F32R)
            nc.sync.dma_start(out=wu_sb, in_=r128(wu_d, 2 * G).bitcast(F32R))
            sb_sb = p1sb.tile([1, 2 * G], F32R)
            nc.sync.dma_start(out=sb_sb, in_=sb_d[:].bitcast(F32R))

            xT_v = r128(xT_d, ROWS)  # [128, 10, ROWS]
            for rc in range(ROWS // 512):
                xT_sb = p1w.tile([128, KF // 128, 512], F32R, tag="xt")
                nc.sync.dma_start(
                    out=xT_sb,
                    in_=xT_v[:, :, rc * 512:(rc + 1) * 512].bitcast(F32R),
                )
                utT_sb = p1w.tile([128, KT, 512], F32R, tag="ut")
                for m in range(KT):
                    psU = p1ps.tile([128, 512], F32, tag="ut", bufs=2)
                    for k in range(KF // 128):
                        nc.tensor.matmul(
                            psU, wf_sb[:, k, m * 128:(m + 1) * 128],
                            xT_sb[:, k, :],
                            start=(k == 0), stop=(k == KF // 128 - 1),
                        )
                    nc.vector.tensor_copy(utT_sb[:, m, :], psU)
                for rt in range(4):
                    psG = p1ps.tile([128, 2 * G], F32, tag="ug", bufs=1)
                    for n in range(2 * G // 512):
                        for k in range(KT):
                            nc.tensor.matmul(
                                psG[:, n * 512:(n + 1) * 512],
                                utT_sb[:, k, rt * 128:(rt + 1) * 128],
                                wu_sb[:, k, n * 512:(n + 1) * 512],
                                start=(k == 0), stop=False,
                            )
                        nc.tensor.matmul(
                            psG[:, n * 512:(n + 1) * 512],
                            ones_col, sb_sb[:, n * 512:(n + 1) * 512],
                            start=False, stop=True,
                        )
                    ug_sb = p1w.tile([128, 2 * G], F32, tag="ugo")
                    nc.vector.tensor_copy(ug_sb, psG)
                    r0 = rc * 512 + rt * 128
                    nc.sync.dma_start(out=ug_d[r0:r0 + 128, :], in_=ug_sb)

        # ---------------- phase 2: weights + scan ----------------
        with ExitStack() as p2:
            wpool = p2.enter_context(tc.tile_pool(name="wpool", bufs=1))
            w_sb = {}
            for nm, dram in wdrams.items():
                w_sb[nm] = wpool.tile([128, KT, G], F32R, name=nm)
                nc.sync.dma_start(out=w_sb[nm],
                                  in_=r128(dram, G).bitcast(F32R))

            io = p2.enter_context(tc.tile_pool(name="io", bufs=1))
            tmp = p2.enter_context(tc.tile_pool(name="tmp", bufs=2))
            ps = p2.enter_context(tc.tile_pool(name="ps", bufs=1, space="PSUM"))

            for t in range(T if RUN_SCAN else 0):
                src = pstA if t % 2 == 0 else pstB
                dst = pstB if t % 2 == 0 else pstA

                ug_t = io.tile([BC, 2 * G], F32, tag="ug", bufs=1)
                nc.sync.dma_start(out=ug_t, in_=ug_d[t * BC:(t + 1) * BC, :])
                rb_t = io.tile([128, P * KT], I16, tag="rb", bufs=2)
                nc.sync.dma_start(out=rb_t, in_=rbidx_d[t])

                # speaker state gather (personal_{t-1}[spk_t]), feature-major
                spT_f = tmp.tile([128, KT, BC], F32, tag="spTf")
                spT = tmp.tile([128, KT, BC], F32R, tag="spT")
                if "gather" in SCAN_PARTS:
                    gix = tmp.tile([128, KT], I16, tag="gix")
                    nc.vector.tensor_copy(gix,
                                          gidx_sb[:, t * KT:(t + 1) * KT])
                    nc.gpsimd.ap_gather(
                        spT_f, src[:, :NSTORE], gix,
                        channels=128, num_elems=NSTORE, d=1, num_idxs=KT * BC,
                    )
                else:
                    nc.vector.tensor_copy(
                        spT_f.rearrange("p k b -> p (k b)"), zro[:, :KT * BC])
                nc.vector.tensor_copy(spT, spT_f)

                # ctx scaling: linv = 1/max(l, 1e-30) broadcast over partitions
                HAS_ATTN = "attn" in SCAN_PARTS
                lm = tmp.tile([1, BC], F32, tag="sm1")
                accS = tmp.tile([128, KT, BC], F32R, tag="accS")
                if HAS_ATTN:
                    nc.vector.tensor_scalar_max(lm, l_sb, 1e-30)
                    linv = tmp.tile([1, BC], F32R, tag="sm2")
                    nc.vector.reciprocal(linv, lm)
                    linv_ps = ps.tile([128, BC], F32, tag="nh", bufs=2)
                    nc.tensor.matmul(linv_ps, ones_col, linv, start=True,
                                     stop=True)
                    linv_bc = tmp.tile([128, BC], F32, tag="lbc")
                    nc.vector.tensor_copy(linv_bc, linv_ps)
                    nc.vector.tensor_tensor(accS, accT, _bcast16(linv_bc),
                                            op=MUL)
                else:
                    nc.vector.tensor_copy(
                        accS.rearrange("p k b -> p (k b)"), zro[:, :KT * BC])

                # global + personal GRU matmuls
                grz = ps.tile([BC, 2, D], F32, tag="rz", bufs=2)
                gni = ps.tile([BC, D], F32, tag="ni", bufs=2)
                gnh = ps.tile([BC, D], F32, tag="nh", bufs=2)
                _mm_gru(nc, grz, gni, gnh, spT, w_sb["wsp"], gT, w_sb["wgh"])
                HAS_P = "p" in SCAN_PARTS
                if HAS_P:
                    prz = ps.tile([BC, 2, D], F32, tag="rz", bufs=2)
                    pni = ps.tile([BC, D], F32, tag="ni", bufs=2)
                    pnh = ps.tile([BC, D], F32, tag="nh", bufs=2)
                    _mm_gru(nc, prz, pni, pnh, accS, w_sb["wpic"], spT,
                            w_sb["wph"])

                # global GRU elementwise -> g_b, gT
                rzg = tmp.tile([BC, 2 * D], F32, tag="rz")
                nc.vector.tensor_add(rzg, grz.rearrange("b n d -> b (n d)"),
                                     ug_t[:, :2 * D])
                nc.scalar.activation(rzg, rzg, AF.Sigmoid)
                t1 = tmp.tile([BC, D], F32, tag="t1")
                nc.vector.tensor_mul(t1, rzg[:, :D], gnh)
                nc.vector.tensor_add(t1, t1, gni)
                nc.vector.tensor_add(t1, t1, ug_t[:, 2 * D:3 * D])
                nc.scalar.activation(t1, t1, AF.Tanh)  # t1 = n
                dd = tmp.tile([BC, D], F32, tag="dd")
                nc.vector.tensor_sub(dd, g_b, t1)
                nc.vector.tensor_mul(dd, dd, rzg[:, D:])
                nc.vector.tensor_add(g_b, dd, t1)
                _transpose_to(nc, ps, ident, g_b, gT)

                if HAS_ATTN:
                    # attention: fold g_t into (m, l, acc)
                    s_ps = ps.tile([1, BC], F32, tag="nh", bufs=2)
                    for k in range(KT):
                        nc.tensor.matmul(s_ps, wa_sb[:, k:k + 1], gT[:, k, :],
                                         start=(k == 0), stop=(k == KT - 1))
                    mn = tmp.tile([1, BC], F32, tag="sm3")
                    nc.vector.tensor_max(mn, m_sb, s_ps)
                    se = tmp.tile([1, 2 * BC], F32R, tag="sm4")
                    d1 = tmp.tile([1, BC], F32, tag="sm5")
                    nc.vector.tensor_sub(d1, m_sb, mn)
                    nc.scalar.activation(se[:, :BC], d1, AF.Exp)
                    d2 = tmp.tile([1, BC], F32, tag="sm6")
                    nc.vector.tensor_sub(d2, s_ps, mn)
                    nc.scalar.activation(se[:, BC:], d2, AF.Exp)
                    nc.vector.tensor_copy(m_sb, mn)
                    nc.vector.tensor_mul(l_sb, l_sb, se[:, :BC])
                    nc.vector.tensor_add(l_sb, l_sb, se[:, BC:])
                    se_ps = ps.tile([128, 2 * BC], F32, tag="nh", bufs=2)
                    nc.tensor.matmul(se_ps, ones_col, se, start=True, stop=True)
                    se_bc = tmp.tile([128, 2 * BC], F32, tag="sebc")
                    nc.vector.tensor_copy(se_bc, se_ps)
                    nc.vector.tensor_tensor(accT, accT, _bcast16(se_bc[:, :BC]),
                                            op=MUL)
                    eg = tmp.tile([128, KT, BC], F32R, tag="eg")
                    nc.vector.tensor_tensor(eg, gT, _bcast16(se_bc[:, BC:]),
                                            op=MUL)
                    nc.vector.tensor_add(accT, accT, eg)

                stg = src[:, NSTORE:].rearrange("p (k b) -> p k b", k=KT)
                if HAS_P:
                    # personal GRU elementwise (h' computed feature-major)
                    rzp = tmp.tile([BC, 2 * D], F32, tag="rz2")
                    nc.vector.tensor_add(rzp,
                                         prz.rearrange("b n d -> b (n d)"),
                                         ug_t[:, G:G + 2 * D])
                    nc.scalar.activation(rzp, rzp, AF.Sigmoid)
                    t2 = tmp.tile([BC, D], F32, tag="t1")
                    nc.vector.tensor_mul(t2, rzp[:, :D], pnh)
                    nc.vector.tensor_add(t2, t2, pni)
                    nc.vector.tensor_add(t2, t2, ug_t[:, G + 2 * D:])
                    nc.scalar.activation(t2, t2, AF.Tanh)  # t2 = n_p
                    zT = tmp.tile([128, KT, BC], F32, tag="zT")
                    _transpose_to(nc, ps, ident, rzp[:, D:], zT)
                    nT = tmp.tile([128, KT, BC], F32, tag="nT")
                    _transpose_to(nc, ps, ident, t2, nT)
                    dT = tmp.tile([128, KT, BC], F32, tag="dT")
                    nc.vector.tensor_sub(dT, spT_f, nT)
                    nc.vector.tensor_mul(dT, dT, zT)
                    nc.vector.tensor_add(stg, dT, nT)

                    # scatter: rebuild store with the speaker column replaced
                    nc.gpsimd.ap_gather(
                        dst[:, :NSTORE], src, rb_t,
                        channels=128, num_elems=NSTORE + KT * BC, d=1,
                        num_idxs=NSTORE,
                    )

                if "e" in SCAN_PARTS:
                    # emotion GRU
                    if HAS_P:
                        stgr = tmp.tile([128, KT, BC], F32R, tag="stgr")
                        nc.vector.tensor_copy(stgr, stg)
                        e_in = stgr
                    else:
                        e_in = spT
                    erz = ps.tile([BC, 2, D], F32, tag="rz", bufs=2)
                    eni = ps.tile([BC, D], F32, tag="ni", bufs=2)
                    enh = ps.tile([BC, D], F32, tag="nh", bufs=2)
                    _mm_gru(nc, erz, eni, enh, e_in, w_sb["wei"], eT,
                            w_sb["weh"])
                    rze = tmp.tile([BC, 2 * D], F32, tag="rz")
                    if add_ebias:
                        nc.vector.tensor_add(
                            rze, erz.rearrange("b n d -> b (n d)"),
                            eb_sb[:, :2 * D])
                        nc.scalar.activation(rze, rze, AF.Sigmoid)
                    else:
                        nc.scalar.activation(
                            rze, erz.rearrange("b n d -> b (n d)"), AF.Sigmoid)
                    t3 = tmp.tile([BC, D], F32, tag="t1")
                    nc.vector.tensor_mul(t3, rze[:, :D], enh)
                    nc.vector.tensor_add(t3, t3, eni)
                    if add_ebias:
                        nc.vector.tensor_add(t3, t3, eb_sb[:, 2 * D:])
                    nc.scalar.activation(t3, t3, AF.Tanh)  # t3 = n_e
                    de = tmp.tile([BC, D], F32, tag="dd")
                    nc.vector.tensor_sub(de, emo_b, t3)
                    nc.vector.tensor_mul(de, de, rze[:, D:])
                    nc.vector.tensor_add(emo_b, de, t3)
                    _transpose_to(nc, ps, ident, emo_b, eT)
                nc.sync.dma_start(out=emo_d[t * BC:(t + 1) * BC, :],
                                  in_=emo_b)

        # ---------------- phase 3: matching-attention head ----------------
        with ExitStack() as p3:
            hw = p3.enter_context(tc.tile_pool(name="hw", bufs=1))
            h3 = p3.enter_context(tc.tile_pool(name="h3", bufs=2))
            ps3 = p3.enter_context(tc.tile_pool(name="ps3", bufs=1,
                                                space="PSUM"))

            wm_sb = hw.tile([128, KT, D], F32R)
            nc.sync.dma_start(out=wm_sb, in_=r128(wm_d, D).bitcast(F32R))
            bm_sb = hw.tile([1, D], F32R)
            nc.sync.dma_start(out=bm_sb, in_=bm_d[:].bitcast(F32R))
            wl_sb = hw.tile([128, KT, D], F32R)
            nc.sync.dma_start(out=wl_sb, in_=r128(wl_d, D).bitcast(F32R))
            bl_sb = hw.tile([1, D], F32R)
            nc.sync.dma_start(out=bl_sb, in_=bl_d[:].bitcast(F32R))
            ws_sb = hw.tile([128, KT, C8], F32R)
            nc.sync.dma_start(out=ws_sb, in_=r128(ws_d, C8).bitcast(F32R))
            bs_sb = hw.tile([1, C8], F32R)
            nc.sync.dma_start(out=bs_sb, in_=bs_d[:].bitcast(F32R))

            TT = T // 128
            emo_v = emo_d.rearrange("(t b) d -> b t d", b=BC)
            out_v = out_d.rearrange("(t b) c -> b t c", b=BC)
            for b in range(BC if RUN_HEAD else 0):
                eb = h3.tile([128, TT, D], F32R, tag="eb")  # [t-part, tt, d]
                nc.sync.dma_start(
                    out=eb,
                    in_=emo_v[b].rearrange("(tt p) d -> p tt d", p=128)
                        .bitcast(F32R),
                )
                ebT = h3.tile([128, KT, T], F32R, tag="ebT")  # [d-part, dc, t]
                for tt in range(TT):
                    trp = ps3.tile([128, 2, 128], F32R, tag="tr", bufs=2)
                    for dc in range(0, KT, 2):
                        for j in range(2):
                            nc.tensor.transpose(
                                trp[:, j, :],
                                eb[:, tt, (dc + j) * 128:(dc + j + 1) * 128],
                                identr,
                            )
                        nc.vector.tensor_copy(
                            ebT[:, dc:dc + 2, tt * 128:(tt + 1) * 128], trp
                        )
                # x_T = Wm @ emo_b.T + bm
                xT3 = h3.tile([128, KT, T], F32R, tag="xT3")
                for m in range(KT):
                    psX = ps3.tile([128, T], F32, tag="mm", bufs=2)
                    for k in range(KT):
                        nc.tensor.matmul(psX, wm_sb[:, k, m * 128:(m + 1) * 128],
                                         ebT[:, k, :], start=(k == 0),
                                         stop=False)
                    nc.tensor.matmul(psX, bm_sb[:, m * 128:(m + 1) * 128],
                                     onesT, start=False, stop=True)
                    nc.vector.tensor_copy(xT3[:, m, :], psX)
                # scores -> tanh -> softmax(al over t)
                al = h3.tile([128, TT, T], F32, tag="al")  # [q-part, qt, t]
                for qt in range(TT):
                    psS = ps3.tile([128, T], F32, tag="mm", bufs=2)
                    for k in range(KT):
                        nc.tensor.matmul(psS, xT3[:, k, qt * 128:(qt + 1) * 128],
                                         ebT[:, k, :], start=(k == 0),
                                         stop=(k == KT - 1))
                    th = h3.tile([128, T], F32, tag="th")
                    nc.scalar.activation(th, psS, AF.Tanh)
                    mx = h3.tile([128, 1], F32, tag="mx")
                    nc.vector.tensor_reduce(mx, th, axis=mybir.AxisListType.X,
                                            op=mybir.AluOpType.max)
                    nc.vector.tensor_scalar_mul(mx, mx, -1.0)
                    ex = h3.tile([128, T], F32, tag="ex")
                    sm = h3.tile([128, 1], F32, tag="sm")
                    nc.scalar.activation(ex, th, AF.Exp, bias=mx, accum_out=sm)
                    nc.vector.reciprocal(sm, sm)
                    nc.vector.tensor_scalar_mul(al[:, qt, :], ex, sm)
                # alT [t-part, tt, q]
                alT = h3.tile([128, TT, T], F32R, tag="alT")
                for qt in range(TT):
                    trp = ps3.tile([128, TT, 128], F32, tag="tr", bufs=2)
                    for tt in range(TT):
                        nc.tensor.transpose(
                            trp[:, tt, :], al[:, qt, tt * 128:(tt + 1) * 128],
                            ident,
                        )
                    nc.vector.tensor_copy(alT[:, :, qt * 128:(qt + 1) * 128],
                                          trp)
                # pooledT [d-part, dc, q] = emo_b.T @ al.T
                pT = h3.tile([128, KT, T], F32R, tag="pT")
                for dc in range(KT):
                    psP = ps3.tile([128, T], F32, tag="mm", bufs=2)
                    for tt in range(TT):
                        nc.tensor.matmul(psP, eb[:, tt, dc * 128:(dc + 1) * 128],
                                         alT[:, tt, :], start=(tt == 0),
                                         stop=(tt == TT - 1))
                    nc.vector.tensor_copy(pT[:, dc, :], psP)
                # hiddenT = relu(Wl @ pooled.T + bl)
                hT = h3.tile([128, KT, T], F32R, tag="hT")
                for m in range(KT):
                    psH = ps3.tile([128, T], F32, tag="mm", bufs=2)
                    for k in range(KT):
                        nc.tensor.matmul(psH, wl_sb[:, k, m * 128:(m + 1) * 128],
                                         pT[:, k, :], start=(k == 0),
                                         stop=False)
                    nc.tensor.matmul(psH, bl_sb[:, m * 128:(m + 1) * 128],
                                     onesT, start=False, stop=True)
                    nc.scalar.activation(hT[:, m, :], psH, AF.Relu)
                # logits + log_softmax
                for qt in range(TT):
                    psL = ps3.tile([128, C8], F32, tag="lg", bufs=2)
                    for k in range(KT):
                        nc.tensor.matmul(psL, hT[:, k, qt * 128:(qt + 1) * 128],
                                         ws_sb[:, k, :], start=(k == 0),
                                         stop=False)
                    nc.tensor.matmul(psL, ones_col, bs_sb, start=False,
                                     stop=True)
                    mx2 = h3.tile([128, 1], F32, tag="mx")
                    nc.vector.tensor_reduce(mx2, psL[:, :C],
                                            axis=mybir.AxisListType.X,
                                            op=mybir.AluOpType.max)
                    nc.vector.tensor_scalar_mul(mx2, mx2, -1.0)
                    ex2 = h3.tile([128, C], F32, tag="ex2")
                    sm2 = h3.tile([128, 1], F32, tag="sm")
                    nc.scalar.activation(ex2, psL[:, :C], AF.Exp, bias=mx2,
                                         accum_out=sm2)
                    nc.scalar.activation(sm2, sm2, AF.Ln)
                    off = h3.tile([128, 1], F32, tag="off")
                    nc.vector.tensor_sub(off, mx2, sm2)
                    lout = h3.tile([128, C], F32, tag="lo")
                    nc.vector.tensor_scalar_add(lout, psL[:, :C], off)
                    nc.sync.dma_start(
                        out=out_v[b, qt * 128:(qt + 1) * 128, :], in_=lout
                    )

    nc.compile()
    return nc


_PROG_CACHE = {}


def kernel(**inputs):
    text = np.asarray(inputs["text"], np.float32)
    video = np.asarray(inputs["video"], np.float32)
    audio = np.asarray(inputs["audio"], np.float32)
    pm = np.asarray(inputs["party_mask"], np.float32)
    mask = np.asarray(inputs["mask"], np.float32)
    Wf, bf = np.asarray(inputs["Wf"]), np.asarray(inputs["bf"])
    Wgi, Wgh = np.asarray(inputs["Wgi"]), np.asarray(inputs["Wgh"])
    bgi, bgh = np.asarray(inputs["bgi"]), np.asarray(inputs["bgh"])
    Wpi, Wph = np.asarray(inputs["Wpi"]), np.asarray(inputs["Wph"])
    bpi, bph = np.asarray(inputs["bpi"]), np.asarray(inputs["bph"])
    Wei, Weh = np.asarray(inputs["Wei"]), np.asarray(inputs["Weh"])
    bei, beh = np.asarray(inputs["bei"]), np.asarray(inputs["beh"])
    w_attn = np.asarray(inputs["w_attn"])
    Wm, bm = np.asarray(inputs["Wm"]), np.asarray(inputs["bm"])
    Wl, bl = np.asarray(inputs["Wl"]), np.asarray(inputs["bl"])
    Ws, bs = np.asarray(inputs["Ws"]), np.asarray(inputs["bs"])

    assert np.all(mask == 1.0), "kernel specialised for all-ones mask"
    spk = np.argmax(pm, axis=2)  # [T, B]
    onehot = np.zeros_like(pm)
    np.put_along_axis(onehot, spk[:, :, None], 1.0, axis=2)
    assert np.array_equal(onehot, pm), "party_mask must be one-hot"

    ebias = (bei + beh).astype(np.float32)
    add_ebias = bool(np.any(ebias != 0.0))

    if add_ebias not in _PROG_CACHE:
        _PROG_CACHE[add_ebias] = build_program(add_ebias)
    nc = _PROG_CACHE[add_ebias]

    # ---- replicated host-side tensor prep ----
    wfe = np.zeros((KF, D), np.float32)
    wfe[:DCAT] = Wf.T
    wfe[DCAT] = bf
    wu = np.concatenate([Wgi[:, :D].T, Wpi[:, :D].T], axis=1)  # [512, 3072]
    sbias = np.concatenate([bgi + bgh, bpi + bph])[None, :].astype(np.float32)
    shared = {
        "wf": wfe,
        "wu": np.ascontiguousarray(wu, dtype=np.float32),
        "sb": sbias,
        "wsp": np.ascontiguousarray(Wgi[:, D:].T, dtype=np.float32),
        "wgh": np.ascontiguousarray(Wgh.T, dtype=np.float32),
        "wpic": np.ascontiguousarray(Wpi[:, D:].T, dtype=np.float32),
        "wph": np.ascontiguousarray(Wph.T, dtype=np.float32),
        "wei": np.ascontiguousarray(Wei.T, dtype=np.float32),
        "weh": np.ascontiguousarray(Weh.T, dtype=np.float32),
        "wa": np.ascontiguousarray(w_attn.reshape(KT, 128).T,
                                   dtype=np.float32),
        "wm": np.ascontiguousarray(Wm.T, dtype=np.float32),
        "bm": bm[None, :].astype(np.float32),
        "wl": np.ascontiguousarray(Wl.T, dtype=np.float32),
        "bl": bl[None, :].astype(np.float32),
        "ws": np.ascontiguousarray(
            np.pad(Ws.T, ((0, 0), (0, C8 - C))), dtype=np.float32),
        "bs": np.pad(bs, (0, C8 - C))[None, :].astype(np.float32),
    }
    if add_ebias:
        shared["ebias"] = ebias[None, :]

    xfull = np.concatenate([text, video, audio], axis=2)  # [T, B, 1200]

    lane = np.arange(BC)
    kk = np.arange(KT)
    party = np.arange(P)
    in_maps = []
    for c in range(NCORES):
        b0 = c * BC
        xs = np.zeros((T * BC, KF), np.float32)
        xs[:, :DCAT] = xfull[:, b0:b0 + BC, :].reshape(T * BC, DCAT)
        xs[:, DCAT] = 1.0
        spk_c = spk[:, b0:b0 + BC]  # [T, BC]

        # ap_gather unwraps idx[j % 16, j // 16] within each 16-partition
        # group; out flat index j = k*16 + b.
        vals = (spk_c[:, :, None] * (KT * BC) + kk[None, None, :] * BC
                + lane[None, :, None])  # [T, BC, KT]
        gidx = np.broadcast_to(
            vals.transpose(1, 0, 2)[None], (8, BC, T, KT)
        ).reshape(128, T * KT).astype(np.int16)

        # rebuild: out flat j = party*64 + k*16 + b -> idx[b, party*4 + k]
        rb = (party[None, :, None] * (KT * BC) + kk[None, None, :] * BC
              + lane[:, None, None])  # [BC, P, KT]
        rb = np.broadcast_to(rb[None], (T, BC, P, KT)).copy()
        stag = (NSTORE + kk[None, None, None, :] * BC
                + lane[None, :, None, None])  # [1, BC, 1, KT]
        is_spk = (party[None, None, :] == spk_c[:, :, None])  # [T, BC, P]
        rb = np.where(is_spk[:, :, :, None], stag, rb)
        rbidx = np.broadcast_to(
            rb.reshape(T, BC, P * KT)[:, None], (T, 8, BC, P * KT)
        ).reshape(T, 128, P * KT).astype(np.int16)

        im = dict(shared)
        im["xT"] = np.ascontiguousarray(xs.T)
        im["gidx"] = np.ascontiguousarray(gidx)
        im["rbidx"] = np.ascontiguousarray(rbidx)
        in_maps.append(im)

    res = run_bass_kernel_spmd(nc, in_maps, list(range(NCORES)))
    outs = [res.results[c]["out"].reshape(T, BC, C) for c in range(NCORES)]
    return np.concatenate(outs, axis=1)



# revision 6
# speedup vs baseline: 1.9930x; 1.9930x over previous
"""DialogueRNN forward on 8 Trainium2 NeuronCores (Bass/Tile, SPMD).

The warm-path cost of this problem is dominated by host->device transfer
over the axon tunnel (~60 MB/s, serialized across cores), so the kernel
is organized around minimizing bytes shipped per call:

  * The trimodal fusion (concat -> Linear, 1200 -> 512) runs on the HOST;
    only the fused utterance tensor crosses the tunnel, in bf16,
    feature-major ([512, T*BC] per core, 4.2 MB instead of 21 MB of raw
    concat input).
  * All weights are packed into one [512, 13320] bf16 matrix, row-sharded
    1/8 per core (1.7 MB each), and reassembled on-device with a DRAM
    AllGather over NeuronLink instead of 8x-replicated host transfers.
  * Index/bias payloads are compacted: speaker gather indices ship as the
    unique [16, T*KT] i16 table (replicated to 128 partitions on device
    with 8 DMAs); the personal-state scatter uses an on-device
    party==speaker mask (is_equal + copy_predicated) built from a tiny
    f32 speaker vector that is partition-broadcast by DMA.

Math structure per core (16 batch lanes), same as the reference:
  1) Ug precompute: ug[r, :] = utter @ [Wgi_u | Wpi_u].T + summed GRU
     biases (ones-row matmul fold), streamed back per scan step.
  2) Sequential scan over T=256 steps: global / personal / emotion
     GRU cells with an online-softmax history attention (m, l, acc kept
     in f32; matmul operands rounded to bf16; the 1/l ctx scale is
     applied to the acc lhsT columns, commuting through the matmul).
     Personal states live feature-major in an SBUF store [128, 9*4*16];
     speaker gather goes through gpsimd.ap_gather, the speaker-only
     update through copy_predicated with a party==spk mask.
  3) MatchingAttention head per batch lane, then
     Linear+ReLU+Linear+log_softmax (f32r).
"""

import sys

sys.path.insert(0, "/opt/trn_rl_repo")

import numpy as np
import ml_dtypes
from contextlib import ExitStack

import concourse.tile as tile
from concourse import bacc
from concourse import mybir
from concourse.bass_utils import run_bass_kernel_spmd
from concourse.masks import make_identity

F32 = mybir.dt.float32
F32R = mybir.dt.float32r
BF16 = mybir.dt.bfloat16
I16 = mybir.dt.int16
NPBF16 = np.dtype(ml_dtypes.bfloat16)
AF = mybir.ActivationFunctionType
MUL = mybir.AluOpType.mult
EQ = mybir.AluOpType.is_equal

T, B, P = 256, 128, 9
Dt, Dv, Da = 600, 300, 300
NCORES = 8
BC = B // NCORES          # 16 batch lanes per core
D = 512                   # Du = Dg = Dp = De = Dh
G = 3 * D                 # 1536 gate width
KT = D // 128             # 4 k-tiles per 512-wide contraction
ROWS = T * BC             # 4096 rows per core
C = 7
C8 = 8                    # class dim padded to 8
NEG = -1e9
NSTORE = P * KT * BC      # 576

# packed-weight column offsets (rows = 512 contraction dim, bf16)
WOFF = {}
_o = 0
for _nm, _w in (("wu", 2 * G), ("wsp", G), ("wgh", G), ("wpic", G),
                ("wph", G), ("wei", G), ("weh", G), ("wm", D), ("wl", D),
                ("ws", C8)):
    WOFF[_nm] = (_o, _w)
    _o += _w
CTOT = _o                 # 13320
WSROWS = D // NCORES      # 64 rows per core shard

# packed small-constant (f32) offsets
SOFF = {}
_o = 0
for _nm, _w in (("sb", 2 * G), ("eb", G), ("bm", D), ("bl", D), ("bs", C8),
                ("spkf", T * BC), ("party", NSTORE)):
    SOFF[_nm] = (_o, _w)
    _o += _w
NS = _o                   # 10312


def _mm_gru(nc, ps_rz, ps_ni, ps_nh, lhsT_i, w_i, lhsT_h, w_h):
    """The 24 matmuls of one GRU step.

    ps_rz [BC, 2, 512]: r,z pre-activations; i-side and h-side accumulate
    into the same banks. ps_ni / ps_nh [BC, 512]: the n-gate parts stay
    separate (n = tanh(i_n + r * h_n)).
    """
    for n in range(2):
        for k in range(KT):
            nc.tensor.matmul(
                ps_rz[:, n, :], lhsT_i[:, k, :], w_i[:, k, n * D:(n + 1) * D],
                start=(k == 0), stop=False,
            )
        for k in range(KT):
            nc.tensor.matmul(
                ps_rz[:, n, :], lhsT_h[:, k, :], w_h[:, k, n * D:(n + 1) * D],
                start=False, stop=(k == KT - 1),
            )
    for k in range(KT):
        nc.tensor.matmul(
            ps_ni, lhsT_i[:, k, :], w_i[:, k, 2 * D:],
            start=(k == 0), stop=(k == KT - 1),
        )
    for k in range(KT):
        nc.tensor.matmul(
            ps_nh, lhsT_h[:, k, :], w_h[:, k, 2 * D:],
            start=(k == 0), stop=(k == KT - 1),
        )


def _transpose_ps(nc, psum_pool, ident, src):
    """src [BC, 512] batch-major -> psum [128, KT, BC] feature-major."""
    trp = psum_pool.tile([128, KT, BC], F32, tag="ni", bufs=2)
    for k in range(KT):
        nc.tensor.transpose(trp[:, k, :], src[:, k * 128:(k + 1) * 128],
                            ident[:BC, :BC])
    return trp


def _bcast16(ap):
    # [128, BC] -> [128, KT, BC] with a stride-0 middle dim
    return ap.rearrange("p (o b) -> p o b", o=1).broadcast_to((128, KT, BC))


def build_program():
    nc = bacc.Bacc("TRN2", target_bir_lowering=False, debug=False,
                   num_devices=NCORES)

    def din(name, shape, dt=F32):
        return nc.dram_tensor(name, shape, dt, kind="ExternalInput").ap()

    ut_d = din("ut", [D, ROWS], BF16)
    wsh_d = din("wsh", [WSROWS, CTOT], BF16)
    sm_d = din("sm", [1, NS])
    wa_d = din("wa", [128, KT], BF16)
    gx_d = din("gx", [BC, T * KT], I16)

    ug_d = nc.dram_tensor("ug_store", [ROWS, 2 * G], F32, kind="Internal").ap()
    emo_d = nc.dram_tensor("emo_store", [ROWS, D], BF16, kind="Internal").ap()
    out_d = nc.dram_tensor("out", [ROWS, C], F32, kind="ExternalOutput").ap()

    def r128(ap, inner):
        # [K*128, inner] DRAM view -> [128, K, inner] partition-major
        return ap.rearrange("(k p) n -> p k n", p=128)

    def smslice(nm):
        o, w = SOFF[nm]
        return sm_d[0:1, o:o + w]

    with ExitStack() as ctx:
        tc = ctx.enter_context(tile.TileContext(nc))
        ctx.enter_context(nc.allow_low_precision(
            reason="deliberate bf16 rounding of matmul operands"))

        # ---------------- weight shard AllGather ----------------
        dram = ctx.enter_context(tc.tile_pool(name="dram", bufs=1,
                                              space="DRAM"))
        wsh_b = dram.tile([WSROWS, CTOT], BF16)
        wfull = dram.tile([D, CTOT], BF16)
        nc.gpsimd.dma_start(out=wsh_b[:], in_=wsh_d[:])
        nc.gpsimd.collective_compute(
            "AllGather", mybir.AluOpType.bypass,
            replica_groups=[list(range(NCORES))],
            ins=[wsh_b.opt()], outs=[wfull.opt()],
        )

        def wslice(nm):
            o, w = WOFF[nm]
            return wfull[:, o:o + w].rearrange("(k p) n -> p k n", p=128)

        const = ctx.enter_context(tc.tile_pool(name="const", bufs=1))
        state = ctx.enter_context(tc.tile_pool(name="state", bufs=1))

        ident = const.tile([128, 128], F32)
        make_identity(nc, ident)
        identr = const.tile([128, 128], F32R)
        nc.vector.tensor_copy(identr, ident)
        ones_f = const.tile([1, max(T, 128)], F32)
        nc.vector.memset(ones_f, 1.0)
        ones_col = const.tile([1, 128], F32R)
        nc.vector.tensor_copy(ones_col, ones_f[:, :128])
        ones_bf = const.tile([1, 128], BF16)
        nc.vector.tensor_copy(ones_bf, ones_f[:, :128])
        onesT = const.tile([1, T], F32R)
        nc.vector.tensor_copy(onesT, ones_f[:, :T])

        wa_sb = const.tile([128, KT], BF16)
        nc.sync.dma_start(out=wa_sb, in_=wa_d[:])
        # speaker gather indices: replicate the unique 16-lane table to all
        # 8 gpsimd partition groups
        gidx_sb = const.tile([128, T * KT], I16)
        for g in range(8):
            nc.sync.dma_start(out=gidx_sb[16 * g:16 * (g + 1), :], in_=gx_d[:])

        eb_sb = const.tile([BC, G], F32)
        nc.sync.dma_start(out=eb_sb, in_=smslice("eb").to_broadcast((BC, G)))
        # speaker id per (t, lane) and party-id pattern, partition-broadcast
        spkf_sb = const.tile([128, T * BC], F32)
        nc.sync.dma_start(out=spkf_sb,
                          in_=smslice("spkf").to_broadcast((128, T * BC)))
        party_sb = const.tile([128, NSTORE], F32)
        nc.sync.dma_start(out=party_sb,
                          in_=smslice("party").to_broadcast((128, NSTORE)))

        # persistent scan state (masters in f32; bf16 copies feed matmuls)
        gT = state.tile([128, KT, BC], BF16)      # global state, feature-major
        gTf = state.tile([128, KT, BC], F32)
        g_b = state.tile([BC, D], F32)            # global state, batch-major
        eT = state.tile([128, KT, BC], BF16)
        emo_b = state.tile([BC, D], F32)
        accT = state.tile([128, KT, BC], F32)
        m_sb = state.tile([1, BC], F32)
        l_sb = state.tile([1, BC], F32)
        pst = state.tile([128, NSTORE], F32)      # personal states store
        for st in (gT, eT):
            nc.vector.memset(st, 0.0)
        for st in (gTf, accT, pst, g_b, emo_b, l_sb):
            nc.vector.memset(st, 0.0)
        nc.vector.memset(m_sb, NEG)

        # ---------------- phases 1+2 share the GRU weight pool ----------
        p12 = ctx.enter_context(ExitStack())
        wpool = p12.enter_context(tc.tile_pool(name="wpool", bufs=1))
        w_sb = {}
        for nm in ("wsp", "wgh", "wpic", "wph", "wei", "weh"):
            w_sb[nm] = wpool.tile([128, KT, G], BF16, name=nm)
            nc.sync.dma_start(out=w_sb[nm], in_=wslice(nm))

        # ---------------- phase 1: Ug precompute ----------------
        with ExitStack() as p1:
            p1w = p1.enter_context(tc.tile_pool(name="p1w", bufs=2))
            p1s = p1.enter_context(tc.tile_pool(name="p1s", bufs=1))
            p1ps = p1.enter_context(tc.tile_pool(name="p1ps", bufs=1,
                                                 space="PSUM"))
            wu_sb = p1s.tile([128, KT, 2 * G], BF16)
            nc.sync.dma_start(out=wu_sb, in_=wslice("wu"))
            sb_f = p1s.tile([1, 2 * G], F32)
            nc.sync.dma_start(out=sb_f, in_=smslice("sb"))
            sb_bf = p1s.tile([1, 2 * G], BF16)
            nc.vector.tensor_copy(sb_bf, sb_f)

            ut_v = r128(ut_d, ROWS)  # [128, KT, ROWS]
            for rc in range(ROWS // 512):
                utT_sb = p1w.tile([128, KT, 512], BF16, tag="ut")
                nc.sync.dma_start(
                    out=utT_sb, in_=ut_v[:, :, rc * 512:(rc + 1) * 512])
                for rt in range(4):
                    psG = p1ps.tile([128, 2 * G], F32, tag="ug", bufs=1)
                    for n in range(2 * G // 512):
                        for k in range(KT):
                            nc.tensor.matmul(
                                psG[:, n * 512:(n + 1) * 512],
                                utT_sb[:, k, rt * 128:(rt + 1) * 128],
                                wu_sb[:, k, n * 512:(n + 1) * 512],
                                start=(k == 0), stop=False,
                            )
                        nc.tensor.matmul(
                            psG[:, n * 512:(n + 1) * 512],
                            ones_bf, sb_bf[:, n * 512:(n + 1) * 512],
                            start=False, stop=True,
                        )
                    ug_sb = p1w.tile([128, 2 * G], F32, tag="ugo")
                    nc.vector.tensor_copy(ug_sb, psG)
                    r0 = rc * 512 + rt * 128
                    nc.sync.dma_start(out=ug_d[r0:r0 + 128, :], in_=ug_sb)

        # ---------------- phase 2: scan ----------------
        with ExitStack() as p2:
            io = p2.enter_context(tc.tile_pool(name="io", bufs=1))
            tmp = p2.enter_context(tc.tile_pool(name="tmp", bufs=2))
            ps = p2.enter_context(tc.tile_pool(name="ps", bufs=1, space="PSUM"))

            for t in range(T):
                ug_t = io.tile([BC, 2 * G], F32, tag="ug", bufs=1)
                nc.sync.dma_start(out=ug_t, in_=ug_d[t * BC:(t + 1) * BC, :])

                # speaker state gather (personal_{t-1}[spk_t]), feature-major
                spT_f = tmp.tile([128, KT, BC], F32, tag="spTf")
                nc.gpsimd.ap_gather(
                    spT_f, pst, gidx_sb[:, t * KT:(t + 1) * KT],
                    channels=128, num_elems=NSTORE, d=1, num_idxs=KT * BC,
                )
                spT = tmp.tile([128, KT, BC], BF16, tag="spT")
                nc.vector.tensor_copy(spT, spT_f)

                # ctx scaling: linv = 1/max(l, 1e-30) broadcast over partitions
                lm = tmp.tile([1, BC], F32, tag="sm1")
                nc.vector.tensor_scalar_max(lm, l_sb, 1e-30)
                linv = tmp.tile([1, BC], F32R, tag="sm2")
                nc.vector.reciprocal(linv, lm)
                linv_ps = ps.tile([128, BC], F32, tag="nh", bufs=2)
                nc.tensor.matmul(linv_ps, ones_col, linv, start=True, stop=True)
                linv_bc = tmp.tile([128, BC], F32, tag="lbc")
                nc.vector.tensor_copy(linv_bc, linv_ps)
                accS = tmp.tile([128, KT, BC], BF16, tag="accS")
                nc.vector.tensor_tensor(accS, accT, _bcast16(linv_bc), op=MUL)

                # global + personal GRU matmuls
                grz = ps.tile([BC, 2, D], F32, tag="rz", bufs=2)
                gni = ps.tile([BC, D], F32, tag="ni", bufs=2)
                gnh = ps.tile([BC, D], F32, tag="nh", bufs=2)
                _mm_gru(nc, grz, gni, gnh, spT, w_sb["wsp"], gT, w_sb["wgh"])
                prz = ps.tile([BC, 2, D], F32, tag="rz", bufs=2)
                pni = ps.tile([BC, D], F32, tag="ni", bufs=2)
                pnh = ps.tile([BC, D], F32, tag="nh", bufs=2)
                _mm_gru(nc, prz, pni, pnh, accS, w_sb["wpic"], spT, w_sb["wph"])

                # global GRU elementwise -> g_b, gT
                rzg = tmp.tile([BC, 2 * D], F32, tag="rz")
                nc.vector.tensor_add(rzg, grz.rearrange("b n d -> b (n d)"),
                                     ug_t[:, :2 * D])
                nc.scalar.activation(rzg, rzg, AF.Sigmoid)
                t1 = tmp.tile([BC, D], F32, tag="t1")
                nc.vector.tensor_mul(t1, rzg[:, :D], gnh)
                nc.vector.tensor_add(t1, t1, gni)
                nc.vector.tensor_add(t1, t1, ug_t[:, 2 * D:3 * D])
                nc.scalar.activation(t1, t1, AF.Tanh)  # t1 = n
                dd = tmp.tile([BC, D], F32, tag="dd")
                nc.vector.tensor_sub(dd, g_b, t1)
                nc.vector.tensor_mul(dd, dd, rzg[:, D:])
                nc.vector.tensor_add(g_b, dd, t1)
                trp = _transpose_ps(nc, ps, ident, g_b)
                nc.vector.tensor_copy(gT, trp)
                nc.vector.tensor_copy(gTf, trp)

                # attention: fold g_t into (m, l, acc)
                s_ps = ps.tile([1, BC], F32, tag="nh", bufs=2)
                for k in range(KT):
                    nc.tensor.matmul(s_ps, wa_sb[:, k:k + 1], gT[:, k, :],
                                     start=(k == 0), stop=(k == KT - 1))
                mn = tmp.tile([1, BC], F32, tag="sm3")
                nc.vector.tensor_max(mn, m_sb, s_ps)
                se = tmp.tile([1, 2 * BC], F32R, tag="sm4")
                d1 = tmp.tile([1, BC], F32, tag="sm5")
                nc.vector.tensor_sub(d1, m_sb, mn)
                nc.scalar.activation(se[:, :BC], d1, AF.Exp)
                d2 = tmp.tile([1, BC], F32, tag="sm6")
                nc.vector.tensor_sub(d2, s_ps, mn)
                nc.scalar.activation(se[:, BC:], d2, AF.Exp)
                nc.vector.tensor_copy(m_sb, mn)
                nc.vector.tensor_mul(l_sb, l_sb, se[:, :BC])
                nc.vector.tensor_add(l_sb, l_sb, se[:, BC:])
                se_ps = ps.tile([128, 2 * BC], F32, tag="nh", bufs=2)
                nc.tensor.matmul(se_ps, ones_col, se, start=True, stop=True)
                se_bc = tmp.tile([128, 2 * BC], F32, tag="sebc")
                nc.vector.tensor_copy(se_bc, se_ps)
                nc.vector.tensor_tensor(accT, accT, _bcast16(se_bc[:, :BC]),
                                        op=MUL)
                eg = tmp.tile([128, KT, BC], F32, tag="eg")
                nc.vector.tensor_tensor(eg, gTf, _bcast16(se_bc[:, BC:]),
                                        op=MUL)
                nc.vector.tensor_add(accT, accT, eg)

                # personal GRU elementwise (h' computed feature-major)
                rzp = tmp.tile([BC, 2 * D], F32, tag="rz2")
                nc.vector.tensor_add(rzp, prz.rearrange("b n d -> b (n d)"),
                                     ug_t[:, G:G + 2 * D])
                nc.scalar.activation(rzp, rzp, AF.Sigmoid)
                t2 = tmp.tile([BC, D], F32, tag="t1")
                nc.vector.tensor_mul(t2, rzp[:, :D], pnh)
                nc.vector.tensor_add(t2, t2, pni)
                nc.vector.tensor_add(t2, t2, ug_t[:, G + 2 * D:])
                nc.scalar.activation(t2, t2, AF.Tanh)  # t2 = n_p
                zT = tmp.tile([128, KT, BC], F32, tag="zT")
                nc.vector.tensor_copy(zT, _transpose_ps(nc, ps, ident,
                                                        rzp[:, D:]))
                nT = tmp.tile([128, KT, BC], F32, tag="nT")
                nc.vector.tensor_copy(nT, _transpose_ps(nc, ps, ident, t2))
                stg = tmp.tile([128, KT, BC], F32, tag="stg")
                dT = tmp.tile([128, KT, BC], F32, tag="dT")
                nc.vector.tensor_sub(dT, spT_f, nT)
                nc.vector.tensor_mul(dT, dT, zT)
                nc.vector.tensor_add(stg, dT, nT)

                # speaker-only store update: mask = (party == spk_t[lane])
                mask = tmp.tile([128, NSTORE], mybir.dt.uint32, tag="msk")
                spk_t = spkf_sb[:, t * BC:(t + 1) * BC]
                nc.vector.tensor_tensor(
                    mask.rearrange("p (j b) -> p j b", b=BC),
                    party_sb.rearrange("p (j b) -> p j b", b=BC),
                    spk_t.rearrange("p (o b) -> p o b", o=1)
                        .broadcast_to((128, P * KT, BC)),
                    op=EQ,
                )
                nc.vector.copy_predicated(
                    pst.rearrange("p (a j) -> p a j", a=P),
                    mask.rearrange("p (a j) -> p a j", a=P),
                    stg.rearrange("p k b -> p (k b)")
                       .rearrange("p (o j) -> p o j", o=1)
                       .broadcast_to((128, P, KT * BC)),
                )

                # emotion GRU (input = updated speaker state)
                e_in = tmp.tile([128, KT, BC], BF16, tag="ein")
                nc.vector.tensor_copy(e_in, stg)
                erz = ps.tile([BC, 2, D], F32, tag="rz", bufs=2)
                eni = ps.tile([BC, D], F32, tag="ni", bufs=2)
                enh = ps.tile([BC, D], F32, tag="nh", bufs=2)
                _mm_gru(nc, erz, eni, enh, e_in, w_sb["wei"], eT, w_sb["weh"])
                rze = tmp.tile([BC, 2 * D], F32, tag="rz")
                nc.vector.tensor_add(rze, erz.rearrange("b n d -> b (n d)"),
                                     eb_sb[:, :2 * D])
                nc.scalar.activation(rze, rze, AF.Sigmoid)
                t3 = tmp.tile([BC, D], F32, tag="t1")
                nc.vector.tensor_mul(t3, rze[:, :D], enh)
                nc.vector.tensor_add(t3, t3, eni)
                nc.vector.tensor_add(t3, t3, eb_sb[:, 2 * D:])
                nc.scalar.activation(t3, t3, AF.Tanh)  # t3 = n_e
                de = tmp.tile([BC, D], F32, tag="dd")
                nc.vector.tensor_sub(de, emo_b, t3)
                nc.vector.tensor_mul(de, de, rze[:, D:])
                nc.vector.tensor_add(emo_b, de, t3)
                nc.vector.tensor_copy(eT, _transpose_ps(nc, ps, ident, emo_b))
                emo16 = tmp.tile([BC, D], BF16, tag="e16")
                nc.vector.tensor_copy(emo16, emo_b)
                nc.sync.dma_start(out=emo_d[t * BC:(t + 1) * BC, :], in_=emo16)

        p12.close()  # free GRU weights before the head

        # ---------------- phase 3: matching-attention head ----------------
        with ExitStack() as p3:
            hw = p3.enter_context(tc.tile_pool(name="hw", bufs=1))
            h3 = p3.enter_context(tc.tile_pool(name="h3", bufs=2))
            ps3 = p3.enter_context(tc.tile_pool(name="ps3", bufs=1,
                                                space="PSUM"))

            def wload(nm, cols):
                bf = hw.tile([128, KT, cols], BF16, name=nm + "bf")
                nc.sync.dma_start(out=bf, in_=wslice(nm))
                fr = hw.tile([128, KT, cols], F32R, name=nm + "fr")
                nc.vector.tensor_copy(fr, bf)
                return fr

            wm_sb = wload("wm", D)
            wl_sb = wload("wl", D)
            ws_sb = wload("ws", C8)
            bm_sb = hw.tile([1, D], F32R)
            nc.sync.dma_start(out=bm_sb, in_=smslice("bm").bitcast(F32R))
            bl_sb = hw.tile([1, D], F32R)
            nc.sync.dma_start(out=bl_sb, in_=smslice("bl").bitcast(F32R))
            bs_sb = hw.tile([1, C8], F32R)
            nc.sync.dma_start(out=bs_sb, in_=smslice("bs").bitcast(F32R))

            TT = T // 128
            emo_v = emo_d.rearrange("(t b) d -> b t d", b=BC)
            out_v = out_d.rearrange("(t b) c -> b t c", b=BC)
            for b in range(BC):
                eb16 = h3.tile([128, TT, D], BF16, tag="eb16")
                nc.sync.dma_start(
                    out=eb16,
                    in_=emo_v[b].rearrange("(tt p) d -> p tt d", p=128),
                )
                eb = h3.tile([128, TT, D], F32R, tag="eb")  # [t-part, tt, d]
                nc.vector.tensor_copy(eb, eb16)
                ebT = h3.tile([128, KT, T], F32R, tag="ebT")  # [d-part, dc, t]
                for tt in range(TT):
                    trp = ps3.tile([128, 2, 128], F32R, tag="tr", bufs=2)
                    for dc in range(0, KT, 2):
                        for j in range(2):
                            nc.tensor.transpose(
                                trp[:, j, :],
                                eb[:, tt, (dc + j) * 128:(dc + j + 1) * 128],
                                identr,
                            )
                        nc.vector.tensor_copy(
                            ebT[:, dc:dc + 2, tt * 128:(tt + 1) * 128], trp
                        )
                # x_T = Wm @ emo_b.T + bm
                xT3 = h3.tile([128, KT, T], F32R, tag="xT3")
                for m in range(KT):
                    psX = ps3.tile([128, T], F32, tag="mm", bufs=2)
                    for k in range(KT):
                        nc.tensor.matmul(psX, wm_sb[:, k, m * 128:(m + 1) * 128],
                                         ebT[:, k, :], start=(k == 0),
                                         stop=False)
                    nc.tensor.matmul(psX, bm_sb[:, m * 128:(m + 1) * 128],
                                     onesT, start=False, stop=True)
                    nc.vector.tensor_copy(xT3[:, m, :], psX)
                # scores -> tanh -> softmax(al over t)
                al = h3.tile([128, TT, T], F32, tag="al")  # [q-part, qt, t]
                for qt in range(TT):
                    psS = ps3.tile([128, T], F32, tag="mm", bufs=2)
                    for k in range(KT):
                        nc.tensor.matmul(psS, xT3[:, k, qt * 128:(qt + 1) * 128],
                                         ebT[:, k, :], start=(k == 0),
                                         stop=(k == KT - 1))
                    th = h3.tile([128, T], F32, tag="th")
                    nc.scalar.activation(th, psS, AF.Tanh)
                    mx = h3.tile([128, 1], F32, tag="mx")
                    nc.vector.tensor_reduce(mx, th, axis=mybir.AxisListType.X,
                                            op=mybir.AluOpType.max)
                    nc.vector.tensor_scalar_mul(mx, mx, -1.0)
                    ex = h3.tile([128, T], F32, tag="ex")
                    sm = h3.tile([128, 1], F32, tag="sm")
                    nc.scalar.activation(ex, th, AF.Exp, bias=mx, accum_out=sm)
                    nc.vector.reciprocal(sm, sm)
                    nc.vector.tensor_scalar_mul(al[:, qt, :], ex, sm)
                # alT [t-part, tt, q]
                alT = h3.tile([128, TT, T], F32R, tag="alT")
                for qt in range(TT):
                    trp = ps3.tile([128, TT, 128], F32, tag="tr", bufs=2)
                    for tt in range(TT):
                        nc.tensor.transpose(
                            trp[:, tt, :], al[:, qt, tt * 128:(tt + 1) * 128],
                            ident,
                        )
                    nc.vector.tensor_copy(alT[:, :, qt * 128:(qt + 1) * 128],
                                          trp)
                # pooledT [d-part, dc, q] = emo_b.T @ al.T
                pT = h3.tile([128, KT, T], F32R, tag="pT")
                for dc in range(KT):
                    psP = ps3.tile([128, T], F32, tag="mm", bufs=2)
                    for tt in range(TT):
                        nc.tensor.matmul(psP, eb[:, tt, dc * 128:(dc + 1) * 128],
                                         alT[:, tt, :], start=(tt == 0),
                                         stop=(tt == TT - 1))
                    nc.vector.tensor_copy(pT[:, dc, :], psP)
                # hiddenT = relu(Wl @ pooled.T + bl)
                hT = h3.tile([128, KT, T], F32R, tag="hT")
                for m in range(KT):
                    psH = ps3.tile([128, T], F32, tag="mm", bufs=2)
                    for k in range(KT):
                        nc.tensor.matmul(psH, wl_sb[:, k, m * 128:(m + 1) * 128],
                                         pT[:, k, :], start=(k == 0),
                                         stop=False)
                    nc.tensor.matmul(psH, bl_sb[:, m * 128:(m + 1) * 128],
                                     onesT, start=False, stop=True)
                    nc.scalar.activation(hT[:, m, :], psH, AF.Relu)
                # logits + log_softmax
                for qt in range(TT):
                    psL = ps3.tile([128, C8], F32, tag="lg", bufs=2)
                    for k in range(KT):
                        nc.tensor.matmul(psL, hT[:, k, qt * 128:(qt + 1) * 128],
                                         ws_sb[:, k, :], start=(k == 0),
                                         stop=False)
                    nc.tensor.matmul(psL, ones_col, bs_sb, start=False,
                                     stop=True)
                    mx2 = h3.tile([128, 1], F32, tag="mx")
                    nc.vector.tensor_reduce(mx2, psL[:, :C],
                                            axis=mybir.AxisListType.X,
                                            op=mybir.AluOpType.max)
                    nc.vector.tensor_scalar_mul(mx2, mx2, -1.0)
                    ex2 = h3.tile([128, C], F32, tag="ex2")
                    sm2 = h3.tile([128, 1], F32, tag="sm")
                    nc.scalar.activation(ex2, psL[:, :C], AF.Exp, bias=mx2,
                                         accum_out=sm2)
                    nc.scalar.activation(sm2, sm2, AF.Ln)
                    off = h3.tile([128, 1], F32, tag="off")
                    nc.vector.tensor_sub(off, mx2, sm2)
                    lout = h3.tile([128, C], F32, tag="lo")
                    nc.vector.tensor_scalar_add(lout, psL[:, :C], off)
                    nc.sync.dma_start(
                        out=out_v[b, qt * 128:(qt + 1) * 128, :], in_=lout
                    )

    nc.compile()
    return nc


_PROG_CACHE = {}


def kernel(**inputs):
    text = np.asarray(inputs["text"], np.float32)
    video = np.asarray(inputs["video"], np.float32)
    audio = np.asarray(inputs["audio"], np.float32)
    pm = np.asarray(inputs["party_mask"], np.float32)
    mask = np.asarray(inputs["mask"], np.float32)
    Wf, bf = np.asarray(inputs["Wf"]), np.asarray(inputs["bf"])
    Wgi, Wgh = np.asarray(inputs["Wgi"]), np.asarray(inputs["Wgh"])
    bgi, bgh = np.asarray(inputs["bgi"]), np.asarray(inputs["bgh"])
    Wpi, Wph = np.asarray(inputs["Wpi"]), np.asarray(inputs["Wph"])
    bpi, bph = np.asarray(inputs["bpi"]), np.asarray(inputs["bph"])
    Wei, Weh = np.asarray(inputs["Wei"]), np.asarray(inputs["Weh"])
    bei, beh = np.asarray(inputs["bei"]), np.asarray(inputs["beh"])
    w_attn = np.asarray(inputs["w_attn"])
    Wm, bm = np.asarray(inputs["Wm"]), np.asarray(inputs["bm"])
    Wl, bl = np.asarray(inputs["Wl"]), np.asarray(inputs["bl"])
    Ws, bs = np.asarray(inputs["Ws"]), np.asarray(inputs["bs"])

    assert np.all(mask == 1.0), "kernel specialised for all-ones mask"
    spk = np.argmax(pm, axis=2)  # [T, B]
    onehot = np.zeros_like(pm)
    np.put_along_axis(onehot, spk[:, :, None], 1.0, axis=2)
    assert np.array_equal(onehot, pm), "party_mask must be one-hot"

    if "prog" not in _PROG_CACHE:
        _PROG_CACHE["prog"] = build_program()
    nc = _PROG_CACHE["prog"]

    # host fusion: utter = concat(text, video, audio) @ Wf.T + bf, done as
    # three GEMMs to avoid materializing the 157MB concat
    utter = (text.reshape(T * B, Dt) @ Wf[:, :Dt].T
             + video.reshape(T * B, Dv) @ Wf[:, Dt:Dt + Dv].T
             + audio.reshape(T * B, Da) @ Wf[:, Dt + Dv:].T
             + bf).reshape(T, NCORES, BC, D)

    # packed bf16 weights [512, CTOT], row-sharded per core
    wpack = np.empty((D, CTOT), np.float32)

    def put(nm, w):
        o, c = WOFF[nm]
        assert w.shape == (D, c), (nm, w.shape)
        wpack[:, o:o + c] = w

    put("wu", np.concatenate([Wgi[:, :D].T, Wpi[:, :D].T], axis=1))
    put("wsp", Wgi[:, D:].T)
    put("wgh", Wgh.T)
    put("wpic", Wpi[:, D:].T)
    put("wph", Wph.T)
    put("wei", Wei.T)
    put("weh", Weh.T)
    put("wm", Wm.T)
    put("wl", Wl.T)
    put("ws", np.pad(Ws.T, ((0, 0), (0, C8 - C))))
    wpack16 = wpack.astype(NPBF16)

    wa16 = np.ascontiguousarray(w_attn.reshape(KT, 128).T).astype(NPBF16)

    smbase = np.zeros(NS, np.float32)

    def sput(nm, v):
        o, c = SOFF[nm]
        assert v.size == c, (nm, v.size)
        smbase[o:o + c] = v.ravel()

    sput("sb", np.concatenate([bgi + bgh, bpi + bph]))
    sput("eb", bei + beh)
    sput("bm", bm)
    sput("bl", bl)
    sput("bs", np.pad(bs, (0, C8 - C)))
    party = np.arange(P)
    sput("party", np.repeat(party, KT * BC))

    lane = np.arange(BC)
    kk = np.arange(KT)
    in_maps = []
    for c in range(NCORES):
        spk_c = spk[:, c * BC:(c + 1) * BC]  # [T, BC]
        # ut: feature-major bf16 [512, T*BC], row r = t*BC + b
        ut = np.ascontiguousarray(
            utter[:, c].reshape(ROWS, D).T).astype(NPBF16)
        # gather indices, unique 16-lane table [BC, T*KT]:
        # idx[lane, t*KT+k] = spk*64 + k*16 + lane  (out element j = k*16+lane)
        gx = (spk_c.T[:, :, None] * (KT * BC) + kk[None, None, :] * BC
              + lane[:, None, None]).reshape(BC, T * KT).astype(np.int16)
        sm = smbase.copy()
        o, w = SOFF["spkf"]
        sm[o:o + w] = spk_c.reshape(ROWS).astype(np.float32)
        in_maps.append({
            "ut": ut,
            "wsh": np.ascontiguousarray(
                wpack16[c * WSROWS:(c + 1) * WSROWS]),
            "sm": sm[None, :],
            "wa": wa16,
            "gx": np.ascontiguousarray(gx),
        })

    res = run_bass_kernel_spmd(nc, in_maps, list(range(NCORES)))
    outs = [res.results[c]["out"].reshape(T, BC, C) for c in range(NCORES)]
    return np.concatenate(outs, axis=1)
